# revision 2
# baseline (speedup 1.0000x reference)
"""Trainium2 Bass kernel for nn_DeepCAD (3x GAT-style GNN + 2-layer BiLSTM + 4 MLP heads).

Strategy: pure data-parallel over batch B=8 across the 8 NeuronCores (one batch
element per core, parameters replicated). Everything runs in a single SPMD NEFF:
  - GNN layers: attention scores built transposed (scoresT[j,i]) so the
    aggregation matmul needs no on-device transpose of the score matrix; the
    row-sum for normalization comes free via an appended ones-column on h.
  - BiLSTM: input projections hoisted (batched matmuls); the 512-step recurrence
    runs with gates on the partition dim; fwd/bwd scans are interleaved so gate
    ALU work of one direction hides under the other's matmuls. Recurrent weights
    in bf16 (fp32 accumulate); everything else fp32.
  - Heads computed in transposed layout; host reassembles (B, T, outd).
"""
import sys
sys.path.insert(0, "/opt/trn_rl_repo")
import numpy as np
from einops import rearrange as rr

import concourse.bass as bass
import concourse.mybir as mybir
import concourse.tile as tile
from concourse import bacc
from concourse.masks import make_identity
from concourse.bass_utils import run_bass_kernel_spmd

F32 = mybir.dt.float32
BF16 = mybir.dt.bfloat16
AF = mybir.ActivationFunctionType

T, B = 512, 8
ND, HD = 128, 256
NUM_OPS, NUM_PARAMS = 64, 16
REC_BF16 = True
PERM = np.r_[0:512, 768:1024, 512:768]  # LSTM gate order [i f o g] from [i f g o]


def _host_prep_weights(inp):
    w = {}
    f32 = lambda a: np.ascontiguousarray(a, dtype=np.float32)
    for li, (pref, ind) in enumerate((("g1", ND), ("g2", HD), ("g3", HD))):
        W = np.asarray(inp[f"{pref}_w"], np.float32)
        b = np.asarray(inp[f"{pref}_b"], np.float32)
        aw = np.asarray(inp[f"{pref}_aw"], np.float32)
        ab = np.asarray(inp[f"{pref}_ab"], np.float32)
        a_i, a_j = aw[0, :HD], aw[0, HD:]
        wa_i, wa_j = a_i @ W, a_j @ W
        c_tot = float(a_i @ b + a_j @ b + ab[0])
        W_aug = np.concatenate([W.T, wa_j[:, None]], 1)
        w[f"L{li}_Waug"] = f32(rr(W_aug, "(c k) n -> k (c n)", k=128))
        w[f"L{li}_baug"] = f32(np.concatenate([b, [c_tot]])[None, :])
        w[f"L{li}_wai"] = f32(rr(np.repeat(wa_i[:, None], 128, 1), "(c k) n -> k (c n)", k=128))
    for s, (l, d) in (("0f", (0, "f")), ("0b", (0, "b")), ("1f", (1, "f")), ("1b", (1, "b"))):
        wih = np.asarray(inp[f"wih{l}{d}"], np.float32)[PERM]
        whh = np.asarray(inp[f"whh{l}{d}"], np.float32)[PERM]
        bb = np.asarray(inp[f"b{l}{d}"], np.float32)[PERM]
        n_kc = wih.shape[1] // 128
        w[f"S{s}_wih"] = f32(rr(wih, "(gc m) (kc k) -> k (kc gc m)", gc=8, kc=n_kc))
        whh_l = rr(whh, "(gc m) (kc k) -> k (kc gc m)", gc=8, kc=2)
        if REC_BF16:
            import ml_dtypes
            w[f"S{s}_whh"] = np.ascontiguousarray(whh_l.astype(ml_dtypes.bfloat16))
        else:
            w[f"S{s}_whh"] = f32(whh_l)
        w[f"S{s}_b"] = f32(bb[None, :])
    for name in ("op", "pp", "sk", "nd"):
        w1 = np.asarray(inp[f"{name}_w1"], np.float32)
        b1 = np.asarray(inp[f"{name}_b1"], np.float32)
        w2 = np.asarray(inp[f"{name}_w2"], np.float32)
        b2 = np.asarray(inp[f"{name}_b2"], np.float32)
        w[f"H{name}_w1"] = f32(rr(w1.T, "(c k) n -> k (c n)", k=128))
        w[f"H{name}_b1"] = f32(b1.reshape(2, 128).T)
        w[f"H{name}_w2"] = f32(rr(w2.T, "(c k) n -> k (c n)", k=128))
        w[f"H{name}_b2"] = f32(b2[:, None])
    return w


def _host_prep_acts(node_features, adjacency, mask, b):
    nf = np.asarray(node_features, np.float32)[b]
    adj = np.asarray(adjacency, np.float32)[b]
    m = np.asarray(mask, np.float32)[b]
    a = {}
    a["nfT"] = np.ascontiguousarray(nf.T)
    adjm = (adj * m[:, None] * m[None, :]).T
    a["adjmT"] = np.ascontiguousarray(rr(adjm, "(c p) i -> p (c i)", p=128))
    a["maskc"] = np.ascontiguousarray(m.reshape(T // 128, 128).T)
    return a


def _build_nc():
    NC = T // 128
    nc = bacc.Bacc("TRN2", target_bir_lowering=False, debug=False, num_devices=8)
    DT_REC = BF16 if REC_BF16 else F32

    dp = lambda n, shp: nc.declare_dram_parameter(n, list(shp), F32, isOutput=False)
    d_in = {}
    d_in["nfT"] = dp("nfT", (128, T))
    d_in["adjmT"] = dp("adjmT", (128, NC * T))
    d_in["maskc"] = dp("maskc", (128, NC))
    for li, ind in ((0, ND), (1, HD), (2, HD)):
        n_kc = ind // 128
        d_in[f"L{li}_Waug"] = dp(f"L{li}_Waug", (128, n_kc * 257))
        d_in[f"L{li}_baug"] = dp(f"L{li}_baug", (1, 257))
        d_in[f"L{li}_wai"] = dp(f"L{li}_wai", (128, n_kc * 128))
    for s, ind in (("0f", HD), ("0b", HD), ("1f", 2 * HD), ("1b", 2 * HD)):
        n_kc = ind // 128
        d_in[f"S{s}_wih"] = dp(f"S{s}_wih", (128, n_kc * 8 * 128))
        d_in[f"S{s}_whh"] = nc.declare_dram_parameter(
            f"S{s}_whh", [128, 2 * 8 * 128], DT_REC, isOutput=False)
        d_in[f"S{s}_b"] = dp(f"S{s}_b", (1, 1024))
    for name, outd in (("op", NUM_OPS), ("pp", NUM_PARAMS), ("sk", 128), ("nd", ND)):
        n_kc = (HD if name == "nd" else 2 * HD) // 128
        d_in[f"H{name}_w1"] = dp(f"H{name}_w1", (128, n_kc * 256))
        d_in[f"H{name}_b1"] = dp(f"H{name}_b1", (128, 2))
        d_in[f"H{name}_w2"] = dp(f"H{name}_w2", (128, 2 * outd))
        d_in[f"H{name}_b2"] = dp(f"H{name}_b2", (outd, 1))
    d_out = {}
    for name, outd in (("op", NUM_OPS), ("pp", NUM_PARAMS), ("sk", 128), ("nd", ND)):
        d_out[name] = nc.declare_dram_parameter(f"o_{name}", [outd, T], F32, isOutput=True)

    with tile.TileContext(nc) as tc:
        import contextlib
        with contextlib.ExitStack() as ctx:
            const = ctx.enter_context(tc.tile_pool(name="const", bufs=1))
            big = ctx.enter_context(tc.tile_pool(name="big", bufs=1))

            ones_row = const.tile([1, T], F32)
            nc.vector.memset(ones_row, 1.0)
            ident = const.tile([128, 128], F32)
            make_identity(nc, ident)
            maskc = const.tile([128, NC], F32)
            nc.sync.dma_start(out=maskc, in_=d_in["maskc"][:])

            hTs = []
            for li in range(3):
                hT_l = big.tile([128, 2, T], F32, tag=f"hT{li}")
                hTs.append(hT_l)

            # ---- GNN ----
            with tc.tile_pool(name="gnnp", bufs=1) as gnnp, \
                 tc.tile_pool(name="gwork", bufs=1) as work, \
                 tc.tile_pool(name="psG", bufs=2, space="PSUM") as psG:
                nfT = gnnp.tile([128, 1, T], F32, tag="xT0")
                nc.sync.dma_start(out=nfT, in_=d_in["nfT"][:].rearrange("k (c t) -> k c t", c=1))
                adjmT = gnnp.tile([128, NC, T], F32)
                nc.sync.dma_start(out=adjmT, in_=d_in["adjmT"][:].rearrange("p (c i) -> p c i", c=NC))

                xT = nfT
                for li, ind in ((0, ND), (1, HD), (2, HD)):
                    n_kc = ind // 128
                    Waug = gnnp.tile([128, n_kc, 257], F32, tag=f"Waug{li}")
                    nc.sync.dma_start(out=Waug, in_=d_in[f"L{li}_Waug"][:].rearrange("k (c n) -> k c n", c=n_kc))
                    baug = gnnp.tile([1, 257], F32, tag=f"baug{li}")
                    nc.sync.dma_start(out=baug, in_=d_in[f"L{li}_baug"][:])
                    wai = gnnp.tile([128, n_kc, 128], F32, tag=f"wai{li}")
                    nc.sync.dma_start(out=wai, in_=d_in[f"L{li}_wai"][:].rearrange("k (c n) -> k c n", c=n_kc))

                    ps_si = psG.tile([128, T], F32, tag="pC")
                    for kc in range(n_kc):
                        nc.tensor.matmul(ps_si, wai[:, kc], xT[:, kc],
                                         start=(kc == 0), stop=(kc == n_kc - 1))
                    h_sb = work.tile([128, NC, 257], F32, tag="h_ext")
                    for nck in range(NC):
                        ps = psG.tile([128, 257], F32, tag="pA")
                        for kc in range(n_kc):
                            nc.tensor.matmul(ps, xT[:, kc, nck * 128:(nck + 1) * 128],
                                             Waug[:, kc], start=(kc == 0), stop=False)
                        nc.tensor.matmul(ps, ones_row[:, nck * 128:(nck + 1) * 128], baug,
                                         start=False, stop=True)
                        nc.scalar.copy(h_sb[:, nck], ps)
                    sc = work.tile([128, NC, T], F32, tag="scoresT")
                    for jc in range(NC):
                        nc.scalar.activation(sc[:, jc], ps_si, AF.Sigmoid,
                                             bias=h_sb[:, jc, 256:257], scale=1.0)
                        nc.vector.tensor_mul(sc[:, jc], sc[:, jc], adjmT[:, jc])
                        nc.vector.memset(h_sb[:, jc, 256:257], 1.0)
                    relu = li < 2
                    h_out = work.tile([128, NC, 256], F32, tag="h_out")
                    for ic in range(NC):
                        ps2 = psG.tile([128, 257], F32, tag="pB")
                        for jc in range(NC):
                            nc.tensor.matmul(ps2, sc[:, jc, ic * 128:(ic + 1) * 128],
                                             h_sb[:, jc], start=(jc == 0), stop=(jc == NC - 1))
                        r = work.tile([128, 1], F32, tag="rnorm")
                        nc.vector.tensor_scalar_add(r, ps2[:, 256:257], 1e-8)
                        nc.vector.reciprocal(r, r)
                        nc.vector.tensor_mul(r, r, maskc[:, ic:ic + 1])
                        nc.scalar.activation(h_out[:, ic], ps2[:, 0:256],
                                             AF.Relu if relu else AF.Identity,
                                             bias=0.0, scale=r)
                    hT = hTs[li]
                    for ic in range(NC):
                        for dc in range(2):
                            pt = psG.tile([128, 128], F32, tag="pA")
                            nc.tensor.transpose(pt, h_out[:, ic, dc * 128:(dc + 1) * 128], ident)
                            nc.scalar.copy(hT[:, dc, ic * 128:(ic + 1) * 128], pt)
                    xT = hT
            h3T = hTs[2]

            # ---- LSTM ----
            with tc.tile_pool(name="lstmp", bufs=1) as lstmp, \
                 tc.tile_pool(name="scanp", bufs=6) as scanp, \
                 tc.tile_pool(name="psL", bufs=2, space="PSUM") as psL, \
                 tc.tile_pool(name="psS", bufs=6, space="PSUM") as psS:

                def load_scan_weights(s, n_kc):
                    wih = lstmp.tile([128, n_kc, 8, 128], F32, tag=f"wih{s}")
                    nc.sync.dma_start(out=wih, in_=d_in[f"S{s}_wih"][:].rearrange(
                        "k (c g m) -> k c g m", c=n_kc, g=8))
                    whh = lstmp.tile([128, 2, 8, 128], DT_REC, tag=f"whh{s}")
                    nc.sync.dma_start(out=whh, in_=d_in[f"S{s}_whh"][:].rearrange(
                        "k (c g m) -> k c g m", c=2, g=8))
                    brow = lstmp.tile([1, 1024], F32, tag=f"b{s}")
                    nc.sync.dma_start(out=brow, in_=d_in[f"S{s}_b"][:])
                    return wih, whh, brow

                def compute_preT(s, n_kc, wih, brow, rhs_chunks):
                    preT = lstmp.tile([128, 8, T], F32, tag=f"pre{s[1]}")
                    for gc in range(8):
                        ps = psL.tile([128, T], F32, tag="pBig")
                        for kc in range(n_kc):
                            nc.tensor.matmul(ps, wih[:, kc, gc], rhs_chunks[kc],
                                             start=(kc == 0), stop=False)
                        nc.tensor.matmul(ps, brow[:, gc * 128:(gc + 1) * 128], ones_row,
                                         start=False, stop=True)
                        if gc % 2 == 0:
                            nc.scalar.copy(preT[:, gc], ps)
                        else:
                            nc.vector.tensor_copy(preT[:, gc], ps)
                    return preT

                def scan_dual(t, lanes):
                    """One time step for both directions, stage-interleaved so the
                    two dependency chains pipeline across PE/ACT/DVE."""
                    gs = []
                    for ln in lanes:
                        tpos, pre, whh, hout, hcast, c_prev = ln
                        if t == 0:
                            g = scanp.tile([128, 8], F32, tag="g")
                            nc.vector.tensor_copy(g, pre[:, :, tpos])
                            gs.append((g, None))
                        else:
                            ps = psS.tile([128, 8], F32, tag="gps")
                            prev_rhs = hcast[0]
                            for gc in range(8):
                                for kc in range(2):
                                    nc.tensor.matmul(ps[:, gc:gc + 1], whh[:, kc, gc],
                                                     prev_rhs[:, kc:kc + 1],
                                                     start=(kc == 0), stop=(kc == 1))
                            gs.append((None, ps))
                    if t != 0:
                        gs2 = []
                        for ln, (_, ps) in zip(lanes, gs):
                            g = scanp.tile([128, 8], F32, tag="g")
                            nc.vector.tensor_add(g, ps, ln[1][:, :, ln[0]])
                            gs2.append((g, None))
                        gs = gs2
                    sigs = []
                    for (g, _) in gs:
                        sig = scanp.tile([128, 6], F32, tag="sig")
                        nc.scalar.activation(sig, g[:, 0:6], AF.Sigmoid)
                        sigs.append(sig)
                    thgs = []
                    for (g, _) in gs:
                        thg = scanp.tile([128, 2], F32, tag="thg")
                        nc.scalar.activation(thg, g[:, 6:8], AF.Tanh)
                        thgs.append(thg)
                    t2s = []
                    for sig, thg in zip(sigs, thgs):
                        t2 = scanp.tile([128, 2], F32, tag="t2")
                        nc.vector.tensor_mul(t2, sig[:, 0:2], thg)
                        t2s.append(t2)
                    cns = []
                    for ln, sig, t2 in zip(lanes, sigs, t2s):
                        if t == 0:
                            cns.append(t2)
                        else:
                            t1 = scanp.tile([128, 2], F32, tag="t1")
                            nc.vector.tensor_mul(t1, sig[:, 2:4], ln[5])
                            c_new = scanp.tile([128, 2], F32, tag="cn")
                            nc.vector.tensor_add(c_new, t1, t2)
                            cns.append(c_new)
                    thcs = []
                    for c_new in cns:
                        thc = scanp.tile([128, 2], F32, tag="thc")
                        nc.scalar.activation(thc, c_new, AF.Tanh)
                        thcs.append(thc)
                    for ln, sig, thc in zip(lanes, sigs, thcs):
                        tpos, pre, whh, hout, hcast = ln[:5]
                        nc.vector.tensor_mul(hout[:, :, tpos], sig[:, 4:6], thc)
                        if REC_BF16:
                            hc = scanp.tile([128, 2], BF16, tag="hc")
                            nc.vector.tensor_copy(hc, hout[:, :, tpos])
                            hcast[0] = hc
                        else:
                            hcast[0] = hout[:, :, tpos]
                    return cns

                def scan_layer(sfx, rhs_chunks_f, n_kc):
                    wih_f, whh_f, b_f = load_scan_weights(sfx + "f", n_kc)
                    wih_b, whh_b, b_b = load_scan_weights(sfx + "b", n_kc)
                    pre_f = compute_preT(sfx + "f", n_kc, wih_f, b_f, rhs_chunks_f)
                    pre_b = compute_preT(sfx + "b", n_kc, wih_b, b_b, rhs_chunks_f)
                    houtf = big.tile([128, 2, T], F32, tag=f"h{sfx}fT")
                    houtb = big.tile([128, 2, T], F32, tag=f"h{sfx}bT")
                    cf, cb = None, None
                    hcf, hcb = [None], [None]
                    for t in range(T):
                        cf, cb = scan_dual(t, [
                            (t, pre_f, whh_f, houtf, hcf, cf),
                            (T - 1 - t, pre_b, whh_b, houtb, hcb, cb)])
                    return houtf, houtb

                h0fT, h0bT = scan_layer("0", [h3T[:, 0], h3T[:, 1]], 2)
                x1_chunks = [h0fT[:, 0], h0fT[:, 1], h0bT[:, 0], h0bT[:, 1]]
                h1fT, h1bT = scan_layer("1", x1_chunks, 4)

            # ---- heads ----
            x2_chunks = [h1fT[:, 0], h1fT[:, 1], h1bT[:, 0], h1bT[:, 1]]
            h3_chunks = [h3T[:, 0], h3T[:, 1]]

            with tc.tile_pool(name="headp", bufs=2) as work2, \
                 tc.tile_pool(name="psH", bufs=2, space="PSUM") as psH:

                def head(name, outd, rhs_chunks):
                    n_kc = len(rhs_chunks)
                    w1 = work2.tile([128, n_kc, 256], F32, tag="hw1")
                    nc.sync.dma_start(out=w1, in_=d_in[f"H{name}_w1"][:].rearrange(
                        "k (c n) -> k c n", c=n_kc))
                    b1 = work2.tile([128, 2], F32, tag="hb1")
                    nc.sync.dma_start(out=b1, in_=d_in[f"H{name}_b1"][:])
                    w2 = work2.tile([128, 2, outd], F32, tag="hw2")
                    nc.sync.dma_start(out=w2, in_=d_in[f"H{name}_w2"][:].rearrange(
                        "k (c n) -> k c n", c=2))
                    b2 = work2.tile([outd, 1], F32, tag="hb2")
                    nc.sync.dma_start(out=b2, in_=d_in[f"H{name}_b2"][:])
                    h1T = work2.tile([128, 2, T], F32, tag="headh1")
                    for dc in range(2):
                        ps = psH.tile([128, T], F32, tag="headps")
                        for kc in range(n_kc):
                            nc.tensor.matmul(ps, w1[:, kc, dc * 128:(dc + 1) * 128],
                                             rhs_chunks[kc], start=(kc == 0), stop=(kc == n_kc - 1))
                        nc.scalar.activation(h1T[:, dc], ps, AF.Relu,
                                             bias=b1[:, dc:dc + 1], scale=1.0)
                    pso = psH.tile([outd, T], F32, tag="headout")
                    for dc in range(2):
                        nc.tensor.matmul(pso, w2[:, dc, :], h1T[:, dc],
                                         start=(dc == 0), stop=(dc == 1))
                    osb = work2.tile([outd, T], F32, tag="headosb")
                    nc.scalar.activation(osb, pso, AF.Identity, bias=b2, scale=1.0)
                    nc.sync.dma_start(out=d_out[name][:], in_=osb)

                head("op", NUM_OPS, x2_chunks)
                head("pp", NUM_PARAMS, x2_chunks)
                head("sk", 128, x2_chunks)
                head("nd", ND, h3_chunks)

    nc.compile()
    return nc


_NC_CACHE = {}


def kernel(**inputs):
    if "nc" not in _NC_CACHE:
        _NC_CACHE["nc"] = _build_nc()
    nc = _NC_CACHE["nc"]
    w = _host_prep_weights(inputs)
    in_maps = [
        {**w, **_host_prep_acts(inputs["node_features"], inputs["adjacency"],
                                inputs["mask"], b)}
        for b in range(B)
    ]
    res = run_bass_kernel_spmd(nc, in_maps, list(range(B)))
    out = []
    for dn in ("o_op", "o_pp", "o_sk", "o_nd"):
        out.append(np.stack([np.ascontiguousarray(res.results[b][dn].T)
                             for b in range(B)]).astype(np.float32))
    return tuple(out)


if __name__ == "__main__":
    rng = np.random.default_rng(0)
    demo = {}
    demo["node_features"] = rng.standard_normal((B, T, ND)).astype(np.float32)
    demo["adjacency"] = (rng.random((B, T, T)) < 0.1).astype(np.float32)
    demo["mask"] = np.ones((B, T), np.float32)
    demo["op_types"] = np.zeros((B, T, NUM_OPS), np.int32)
    nrm = lambda shape: (rng.standard_normal(shape) * 0.02).astype(np.float32)
    for i, ind in zip((1, 2, 3), (ND, HD, HD)):
        demo[f"g{i}_w"] = nrm((HD, ind)); demo[f"g{i}_b"] = nrm((HD,))
        demo[f"g{i}_aw"] = nrm((1, 2 * HD)); demo[f"g{i}_ab"] = nrm((1,))
    for l, ind in ((0, HD), (1, 2 * HD)):
        for d in "fb":
            demo[f"wih{l}{d}"] = nrm((4 * HD, ind))
            demo[f"whh{l}{d}"] = nrm((4 * HD, HD))
            demo[f"b{l}{d}"] = nrm((4 * HD,))
    for name, outd in (("op", NUM_OPS), ("pp", NUM_PARAMS), ("sk", 128), ("nd", ND)):
        ind = HD if name == "nd" else 2 * HD
        demo[f"{name}_w1"] = nrm((HD, ind)); demo[f"{name}_b1"] = nrm((HD,))
        demo[f"{name}_w2"] = nrm((outd, HD)); demo[f"{name}_b2"] = nrm((outd,))
    outs = kernel(**demo)
    print([o.shape for o in outs])


# revision 3
# speedup vs baseline: 1.0395x; 1.0395x over previous
"""Trainium2 Bass kernel for nn_DeepCAD (3x GAT-style GNN + 2-layer BiLSTM + 4 MLP heads).

Strategy: pure data-parallel over batch B=8 across the 8 NeuronCores (one batch
element per core, parameters replicated). Everything runs in a single SPMD NEFF:
  - GNN layers: attention scores built transposed (scoresT[j,i]) so the
    aggregation matmul needs no on-device transpose of the score matrix; the
    row-sum for normalization comes free via an appended ones-column on h.
  - BiLSTM: input projections hoisted (batched matmuls); the 512-step recurrence
    runs with gates on the partition dim; fwd/bwd scans are interleaved so gate
    ALU work of one direction hides under the other's matmuls. Recurrent weights
    in bf16 (fp32 accumulate); everything else fp32.
  - Heads computed in transposed layout; host reassembles (B, T, outd).
"""
import sys
sys.path.insert(0, "/opt/trn_rl_repo")
import numpy as np
from einops import rearrange as rr

import concourse.bass as bass
import concourse.mybir as mybir
import concourse.tile as tile
from concourse import bacc
from concourse.masks import make_identity
from concourse.bass_utils import run_bass_kernel_spmd

F32 = mybir.dt.float32
BF16 = mybir.dt.bfloat16
AF = mybir.ActivationFunctionType

T, B = 512, 8
ND, HD = 128, 256
NUM_OPS, NUM_PARAMS = 64, 16
REC_BF16 = True
PERM = np.r_[0:512, 768:1024, 512:768]  # LSTM gate order [i f o g] from [i f g o]


def _host_prep_weights(inp):
    w = {}
    f32 = lambda a: np.ascontiguousarray(a, dtype=np.float32)
    for li, (pref, ind) in enumerate((("g1", ND), ("g2", HD), ("g3", HD))):
        W = np.asarray(inp[f"{pref}_w"], np.float32)
        b = np.asarray(inp[f"{pref}_b"], np.float32)
        aw = np.asarray(inp[f"{pref}_aw"], np.float32)
        ab = np.asarray(inp[f"{pref}_ab"], np.float32)
        a_i, a_j = aw[0, :HD], aw[0, HD:]
        wa_i, wa_j = a_i @ W, a_j @ W
        c_tot = float(a_i @ b + a_j @ b + ab[0])
        W_aug = np.concatenate([W.T, wa_j[:, None]], 1)
        w[f"L{li}_Waug"] = f32(rr(W_aug, "(c k) n -> k (c n)", k=128))
        w[f"L{li}_baug"] = f32(np.concatenate([b, [c_tot]])[None, :])
        w[f"L{li}_wai"] = f32(rr(np.repeat(wa_i[:, None], 128, 1), "(c k) n -> k (c n)", k=128))
    for s, (l, d) in (("0f", (0, "f")), ("0b", (0, "b")), ("1f", (1, "f")), ("1b", (1, "b"))):
        wih = np.asarray(inp[f"wih{l}{d}"], np.float32)[PERM]
        whh = np.asarray(inp[f"whh{l}{d}"], np.float32)[PERM]
        bb = np.asarray(inp[f"b{l}{d}"], np.float32)[PERM]
        n_kc = wih.shape[1] // 128
        wih_l = rr(wih, "(gc m) (kc k) -> k (kc gc m)", gc=8, kc=n_kc)
        if REC_BF16 and l == 1:
            import ml_dtypes
            w[f"S{s}_wih"] = np.ascontiguousarray(wih_l.astype(ml_dtypes.bfloat16))
        else:
            w[f"S{s}_wih"] = f32(wih_l)
        whh_l = rr(whh, "(gc m) (kc k) -> k (kc gc m)", gc=8, kc=2)
        if REC_BF16:
            import ml_dtypes
            w[f"S{s}_whh"] = np.ascontiguousarray(whh_l.astype(ml_dtypes.bfloat16))
        else:
            w[f"S{s}_whh"] = f32(whh_l)
        w[f"S{s}_b"] = f32(bb[None, :])
    for name in ("op", "pp", "sk", "nd"):
        w1 = np.asarray(inp[f"{name}_w1"], np.float32)
        b1 = np.asarray(inp[f"{name}_b1"], np.float32)
        w2 = np.asarray(inp[f"{name}_w2"], np.float32)
        b2 = np.asarray(inp[f"{name}_b2"], np.float32)
        w1_l = rr(w1.T, "(c k) n -> k (c n)", k=128)
        if REC_BF16 and name != "nd":
            import ml_dtypes
            w[f"H{name}_w1"] = np.ascontiguousarray(w1_l.astype(ml_dtypes.bfloat16))
        else:
            w[f"H{name}_w1"] = f32(w1_l)
        w[f"H{name}_b1"] = f32(b1.reshape(2, 128).T)
        w[f"H{name}_w2"] = f32(rr(w2.T, "(c k) n -> k (c n)", k=128))
        w[f"H{name}_b2"] = f32(b2[:, None])
    return w


def _host_prep_acts(node_features, adjacency, mask, b):
    nf = np.asarray(node_features, np.float32)[b]
    adj = np.asarray(adjacency, np.float32)[b]
    m = np.asarray(mask, np.float32)[b]
    a = {}
    a["nfT"] = np.ascontiguousarray(nf.T)
    adjm = (adj * m[:, None] * m[None, :]).T
    a["adjmT"] = np.ascontiguousarray(rr(adjm, "(c p) i -> p (c i)", p=128))
    a["maskc"] = np.ascontiguousarray(m.reshape(T // 128, 128).T)
    return a


def _build_nc():
    NC = T // 128
    nc = bacc.Bacc("TRN2", target_bir_lowering=False, debug=False, num_devices=8)
    DT_REC = BF16 if REC_BF16 else F32

    dp = lambda n, shp: nc.declare_dram_parameter(n, list(shp), F32, isOutput=False)
    d_in = {}
    d_in["nfT"] = dp("nfT", (128, T))
    d_in["adjmT"] = dp("adjmT", (128, NC * T))
    d_in["maskc"] = dp("maskc", (128, NC))
    for li, ind in ((0, ND), (1, HD), (2, HD)):
        n_kc = ind // 128
        d_in[f"L{li}_Waug"] = dp(f"L{li}_Waug", (128, n_kc * 257))
        d_in[f"L{li}_baug"] = dp(f"L{li}_baug", (1, 257))
        d_in[f"L{li}_wai"] = dp(f"L{li}_wai", (128, n_kc * 128))
    for s, ind in (("0f", HD), ("0b", HD), ("1f", 2 * HD), ("1b", 2 * HD)):
        n_kc = ind // 128
        wih_dt = DT_REC if s[0] == "1" else F32
        d_in[f"S{s}_wih"] = nc.declare_dram_parameter(
            f"S{s}_wih", [128, n_kc * 8 * 128], wih_dt, isOutput=False)
        d_in[f"S{s}_whh"] = nc.declare_dram_parameter(
            f"S{s}_whh", [128, 2 * 8 * 128], DT_REC, isOutput=False)
        d_in[f"S{s}_b"] = dp(f"S{s}_b", (1, 1024))
    for name, outd in (("op", NUM_OPS), ("pp", NUM_PARAMS), ("sk", 128), ("nd", ND)):
        n_kc = (HD if name == "nd" else 2 * HD) // 128
        hw1_dt = F32 if name == "nd" else DT_REC
        d_in[f"H{name}_w1"] = nc.declare_dram_parameter(
            f"H{name}_w1", [128, n_kc * 256], hw1_dt, isOutput=False)
        d_in[f"H{name}_b1"] = dp(f"H{name}_b1", (128, 2))
        d_in[f"H{name}_w2"] = dp(f"H{name}_w2", (128, 2 * outd))
        d_in[f"H{name}_b2"] = dp(f"H{name}_b2", (outd, 1))
    d_out = {}
    for name, outd in (("op", NUM_OPS), ("pp", NUM_PARAMS), ("sk", 128), ("nd", ND)):
        d_out[name] = nc.declare_dram_parameter(f"o_{name}", [outd, T], F32, isOutput=True)

    with tile.TileContext(nc) as tc:
        import contextlib
        with contextlib.ExitStack() as ctx:
            const = ctx.enter_context(tc.tile_pool(name="const", bufs=1))
            big = ctx.enter_context(tc.tile_pool(name="big", bufs=1))

            ones_row = const.tile([1, T], F32)
            nc.vector.memset(ones_row, 1.0)
            ident = const.tile([128, 128], F32)
            make_identity(nc, ident)
            maskc = const.tile([128, NC], F32)
            nc.sync.dma_start(out=maskc, in_=d_in["maskc"][:])

            hTs = []
            for li in range(3):
                hT_l = big.tile([128, 2, T], F32, tag=f"hT{li}")
                hTs.append(hT_l)

            # ---- GNN ----
            with tc.tile_pool(name="gnnp", bufs=1) as gnnp, \
                 tc.tile_pool(name="gwork", bufs=1) as work, \
                 tc.tile_pool(name="psG", bufs=2, space="PSUM") as psG:
                nfT = gnnp.tile([128, 1, T], F32, tag="xT0")
                nc.sync.dma_start(out=nfT, in_=d_in["nfT"][:].rearrange("k (c t) -> k c t", c=1))
                adjmT = gnnp.tile([128, NC, T], F32)
                nc.sync.dma_start(out=adjmT, in_=d_in["adjmT"][:].rearrange("p (c i) -> p c i", c=NC))

                xT = nfT
                for li, ind in ((0, ND), (1, HD), (2, HD)):
                    n_kc = ind // 128
                    Waug = gnnp.tile([128, n_kc, 257], F32, tag=f"Waug{li}")
                    nc.sync.dma_start(out=Waug, in_=d_in[f"L{li}_Waug"][:].rearrange("k (c n) -> k c n", c=n_kc))
                    baug = gnnp.tile([1, 257], F32, tag=f"baug{li}")
                    nc.sync.dma_start(out=baug, in_=d_in[f"L{li}_baug"][:])
                    wai = gnnp.tile([128, n_kc, 128], F32, tag=f"wai{li}")
                    nc.sync.dma_start(out=wai, in_=d_in[f"L{li}_wai"][:].rearrange("k (c n) -> k c n", c=n_kc))

                    ps_si = psG.tile([128, T], F32, tag="pC")
                    for kc in range(n_kc):
                        nc.tensor.matmul(ps_si, wai[:, kc], xT[:, kc],
                                         start=(kc == 0), stop=(kc == n_kc - 1))
                    h_sb = work.tile([128, NC, 257], F32, tag="h_ext")
                    for nck in range(NC):
                        ps = psG.tile([128, 257], F32, tag="pA")
                        for kc in range(n_kc):
                            nc.tensor.matmul(ps, xT[:, kc, nck * 128:(nck + 1) * 128],
                                             Waug[:, kc], start=(kc == 0), stop=False)
                        nc.tensor.matmul(ps, ones_row[:, nck * 128:(nck + 1) * 128], baug,
                                         start=False, stop=True)
                        nc.scalar.copy(h_sb[:, nck], ps)
                    sc = work.tile([128, NC, T], F32, tag="scoresT")
                    for jc in range(NC):
                        nc.scalar.activation(sc[:, jc], ps_si, AF.Sigmoid,
                                             bias=h_sb[:, jc, 256:257], scale=1.0)
                        nc.vector.tensor_mul(sc[:, jc], sc[:, jc], adjmT[:, jc])
                        nc.vector.memset(h_sb[:, jc, 256:257], 1.0)
                    relu = li < 2
                    h_out = work.tile([128, NC, 256], F32, tag="h_out")
                    for ic in range(NC):
                        ps2 = psG.tile([128, 257], F32, tag="pB")
                        for jc in range(NC):
                            nc.tensor.matmul(ps2, sc[:, jc, ic * 128:(ic + 1) * 128],
                                             h_sb[:, jc], start=(jc == 0), stop=(jc == NC - 1))
                        r = work.tile([128, 1], F32, tag="rnorm")
                        nc.vector.tensor_scalar_add(r, ps2[:, 256:257], 1e-8)
                        nc.vector.reciprocal(r, r)
                        nc.vector.tensor_mul(r, r, maskc[:, ic:ic + 1])
                        nc.scalar.activation(h_out[:, ic], ps2[:, 0:256],
                                             AF.Relu if relu else AF.Identity,
                                             bias=0.0, scale=r)
                    hT = hTs[li]
                    for ic in range(NC):
                        for dc in range(2):
                            pt = psG.tile([128, 128], F32, tag="pA")
                            nc.tensor.transpose(pt, h_out[:, ic, dc * 128:(dc + 1) * 128], ident)
                            nc.scalar.copy(hT[:, dc, ic * 128:(ic + 1) * 128], pt)
                    xT = hT
            h3T = hTs[2]

            # ---- LSTM ----
            with tc.tile_pool(name="lstmp", bufs=1) as lstmp, \
                 tc.tile_pool(name="scanp", bufs=6) as scanp, \
                 tc.tile_pool(name="psL", bufs=2, space="PSUM") as psL, \
                 tc.tile_pool(name="psS", bufs=6, space="PSUM") as psS:

                def load_scan_weights(s, n_kc):
                    wih_dt = DT_REC if s[0] == "1" else F32
                    wih = lstmp.tile([128, n_kc, 8, 128], wih_dt, tag=f"wih{s}")
                    nc.sync.dma_start(out=wih, in_=d_in[f"S{s}_wih"][:].rearrange(
                        "k (c g m) -> k c g m", c=n_kc, g=8))
                    whh = lstmp.tile([128, 2, 8, 128], DT_REC, tag=f"whh{s}")
                    nc.sync.dma_start(out=whh, in_=d_in[f"S{s}_whh"][:].rearrange(
                        "k (c g m) -> k c g m", c=2, g=8))
                    brow = lstmp.tile([1, 1024], F32, tag=f"b{s}")
                    nc.sync.dma_start(out=brow, in_=d_in[f"S{s}_b"][:])
                    return wih, whh, brow

                def compute_preT(s, n_kc, wih, brow, rhs_chunks):
                    preT = lstmp.tile([128, 8, T], F32, tag=f"pre{s[1]}")
                    for gc in range(8):
                        ps = psL.tile([128, T], F32, tag="pBig")
                        for kc in range(n_kc):
                            nc.tensor.matmul(ps, wih[:, kc, gc], rhs_chunks[kc],
                                             start=(kc == 0), stop=False)
                        nc.tensor.matmul(ps, brow[:, gc * 128:(gc + 1) * 128], ones_row,
                                         start=False, stop=True)
                        if gc % 2 == 0:
                            nc.scalar.copy(preT[:, gc], ps)
                        else:
                            nc.vector.tensor_copy(preT[:, gc], ps)
                    return preT

                def scan_dual(t, tf, tb, pre_f, pre_b, whh_f, whh_b,
                              houtf, houtb, c_fb_prev):
                    """One time step, both directions; gate math merged into
                    shared tiles (f = lane 0, b = lane 1)."""
                    if t == 0:
                        g_fb = scanp.tile([128, 2, 8], F32, tag="g")
                        nc.vector.tensor_copy(g_fb[:, 0], pre_f[:, :, tf])
                        nc.vector.tensor_copy(g_fb[:, 1], pre_b[:, :, tb])
                    else:
                        ps = psS.tile([128, 2, 8], F32, tag="gps")
                        for lane, (whh, hout, tprev) in enumerate(
                                ((whh_f, houtf, tf - 1), (whh_b, houtb, tb + 1))):
                            prev_rhs = hout[:, :, tprev]
                            for gc in range(8):
                                for kc in range(2):
                                    nc.tensor.matmul(ps[:, lane, gc:gc + 1],
                                                     whh[:, kc, gc],
                                                     prev_rhs[:, kc:kc + 1],
                                                     start=(kc == 0), stop=(kc == 1))
                        g_fb = scanp.tile([128, 2, 8], F32, tag="g")
                        nc.vector.tensor_add(g_fb[:, 0], ps[:, 0], pre_f[:, :, tf])
                        nc.vector.tensor_add(g_fb[:, 1], ps[:, 1], pre_b[:, :, tb])
                    sig = scanp.tile([128, 2, 6], F32, tag="sig")
                    nc.scalar.activation(sig, g_fb[:, :, 0:6], AF.Sigmoid)
                    thg = scanp.tile([128, 2, 2], F32, tag="thg")
                    nc.scalar.activation(thg, g_fb[:, :, 6:8], AF.Tanh)
                    t2 = scanp.tile([128, 2, 2], F32, tag="t2")
                    nc.vector.tensor_mul(t2, sig[:, :, 0:2], thg)
                    if t == 0:
                        c_fb = t2
                    else:
                        t1 = scanp.tile([128, 2, 2], F32, tag="t1")
                        nc.vector.tensor_mul(t1, sig[:, :, 2:4], c_fb_prev)
                        c_fb = scanp.tile([128, 2, 2], F32, tag="cn")
                        nc.vector.tensor_add(c_fb, t1, t2)
                    thc = scanp.tile([128, 2, 2], F32, tag="thc")
                    nc.scalar.activation(thc, c_fb, AF.Tanh)
                    nc.vector.tensor_mul(houtf[:, :, tf], sig[:, 0, 4:6], thc[:, 0])
                    nc.vector.tensor_mul(houtb[:, :, tb], sig[:, 1, 4:6], thc[:, 1])
                    return c_fb

                def scan_layer(sfx, rhs_chunks_f, n_kc):
                    wih_f, whh_f, b_f = load_scan_weights(sfx + "f", n_kc)
                    wih_b, whh_b, b_b = load_scan_weights(sfx + "b", n_kc)
                    pre_f = compute_preT(sfx + "f", n_kc, wih_f, b_f, rhs_chunks_f)
                    pre_b = compute_preT(sfx + "b", n_kc, wih_b, b_b, rhs_chunks_f)
                    houtf = big.tile([128, 2, T], DT_REC, tag=f"h{sfx}fT")
                    houtb = big.tile([128, 2, T], DT_REC, tag=f"h{sfx}bT")
                    c_fb = None
                    for t in range(T):
                        c_fb = scan_dual(t, t, T - 1 - t, pre_f, pre_b,
                                         whh_f, whh_b, houtf, houtb, c_fb)
                    return houtf, houtb

                h0fT, h0bT = scan_layer("0", [h3T[:, 0], h3T[:, 1]], 2)
                x1_chunks = [h0fT[:, 0], h0fT[:, 1], h0bT[:, 0], h0bT[:, 1]]
                h1fT, h1bT = scan_layer("1", x1_chunks, 4)

            # ---- heads ----
            x2_chunks = [h1fT[:, 0], h1fT[:, 1], h1bT[:, 0], h1bT[:, 1]]
            h3_chunks = [h3T[:, 0], h3T[:, 1]]

            with tc.tile_pool(name="headp", bufs=2) as work2, \
                 tc.tile_pool(name="psH", bufs=2, space="PSUM") as psH:

                def head(name, outd, rhs_chunks):
                    n_kc = len(rhs_chunks)
                    w1_dt = F32 if name == "nd" else DT_REC
                    w1 = work2.tile([128, n_kc, 256], w1_dt, tag="hw1")
                    nc.sync.dma_start(out=w1, in_=d_in[f"H{name}_w1"][:].rearrange(
                        "k (c n) -> k c n", c=n_kc))
                    b1 = work2.tile([128, 2], F32, tag="hb1")
                    nc.sync.dma_start(out=b1, in_=d_in[f"H{name}_b1"][:])
                    w2 = work2.tile([128, 2, outd], F32, tag="hw2")
                    nc.sync.dma_start(out=w2, in_=d_in[f"H{name}_w2"][:].rearrange(
                        "k (c n) -> k c n", c=2))
                    b2 = work2.tile([outd, 1], F32, tag="hb2")
                    nc.sync.dma_start(out=b2, in_=d_in[f"H{name}_b2"][:])
                    h1T = work2.tile([128, 2, T], F32, tag="headh1")
                    for dc in range(2):
                        ps = psH.tile([128, T], F32, tag="headps")
                        for kc in range(n_kc):
                            nc.tensor.matmul(ps, w1[:, kc, dc * 128:(dc + 1) * 128],
                                             rhs_chunks[kc], start=(kc == 0), stop=(kc == n_kc - 1))
                        nc.scalar.activation(h1T[:, dc], ps, AF.Relu,
                                             bias=b1[:, dc:dc + 1], scale=1.0)
                    pso = psH.tile([outd, T], F32, tag="headout")
                    for dc in range(2):
                        nc.tensor.matmul(pso, w2[:, dc, :], h1T[:, dc],
                                         start=(dc == 0), stop=(dc == 1))
                    osb = work2.tile([outd, T], F32, tag="headosb")
                    nc.scalar.activation(osb, pso, AF.Identity, bias=b2, scale=1.0)
                    nc.sync.dma_start(out=d_out[name][:], in_=osb)

                head("op", NUM_OPS, x2_chunks)
                head("pp", NUM_PARAMS, x2_chunks)
                head("sk", 128, x2_chunks)
                head("nd", ND, h3_chunks)

    nc.compile()
    return nc


_NC_CACHE = {}


def kernel(**inputs):
    if "nc" not in _NC_CACHE:
        _NC_CACHE["nc"] = _build_nc()
    nc = _NC_CACHE["nc"]
    w = _host_prep_weights(inputs)
    in_maps = [
        {**w, **_host_prep_acts(inputs["node_features"], inputs["adjacency"],
                                inputs["mask"], b)}
        for b in range(B)
    ]
    res = run_bass_kernel_spmd(nc, in_maps, list(range(B)))
    out = []
    for dn in ("o_op", "o_pp", "o_sk", "o_nd"):
        out.append(np.stack([np.ascontiguousarray(res.results[b][dn].T)
                             for b in range(B)]).astype(np.float32))
    return tuple(out)


if __name__ == "__main__":
    rng = np.random.default_rng(0)
    demo = {}
    demo["node_features"] = rng.standard_normal((B, T, ND)).astype(np.float32)
    demo["adjacency"] = (rng.random((B, T, T)) < 0.1).astype(np.float32)
    demo["mask"] = np.ones((B, T), np.float32)
    demo["op_types"] = np.zeros((B, T, NUM_OPS), np.int32)
    nrm = lambda shape: (rng.standard_normal(shape) * 0.02).astype(np.float32)
    for i, ind in zip((1, 2, 3), (ND, HD, HD)):
        demo[f"g{i}_w"] = nrm((HD, ind)); demo[f"g{i}_b"] = nrm((HD,))
        demo[f"g{i}_aw"] = nrm((1, 2 * HD)); demo[f"g{i}_ab"] = nrm((1,))
    for l, ind in ((0, HD), (1, 2 * HD)):
        for d in "fb":
            demo[f"wih{l}{d}"] = nrm((4 * HD, ind))
            demo[f"whh{l}{d}"] = nrm((4 * HD, HD))
            demo[f"b{l}{d}"] = nrm((4 * HD,))
    for name, outd in (("op", NUM_OPS), ("pp", NUM_PARAMS), ("sk", 128), ("nd", ND)):
        ind = HD if name == "nd" else 2 * HD
        demo[f"{name}_w1"] = nrm((HD, ind)); demo[f"{name}_b1"] = nrm((HD,))
        demo[f"{name}_w2"] = nrm((outd, HD)); demo[f"{name}_b2"] = nrm((outd,))
    outs = kernel(**demo)
    print([o.shape for o in outs])
